# revision 20
# baseline (speedup 1.0000x reference)
"""ContraNorm Trainium2 kernel (8 NeuronCores, fp8 DoubleRow, flash-style).

Reference computation (N=16384, D=256, f32):
    x_norm = x / max(||x||_row, 1e-12)
    sim    = softmax(x_norm @ x_norm.T, axis=1)
    out    = 1.1 * x - 0.1 * (sim @ x)

Sharding: row-parallel, no collectives. Core c receives x ROLLED so its own
2048 rows sit first; row-softmax and the P-accumulation are permutation
invariant over n, so rolling is transparent. Own rows = first M rows.

Per-core algorithm (all matmuls fp8e4 DoubleRow = 2x PE throughput; the two
128-row k-subtiles ride the byte-pair interleave produced by a uint16 DMA
transpose of packed fp8 pairs, so contraction index d = b*128 + p on both
operands automatically):
  setup (per 8-chunk group, pipelined with main):
    xt   = dma(x)                              f32 [128, 8, 256]
    xa   = fp8(x) pair-interleaved + [1,1]     [128, nch, 258] (gpsimd cast)
    ssq  = sum_d x^2 (ACT Square + accum), r16 = 16/||x|| = exp(-.5 ln ssq + ln16)
    xnT  = dma_transpose(xa chunk as u16)      compact [128, 2*n] fp8 raw x
    xnT *= r (broadcast over partitions)       => 16 * x_norm^T, fp8
  main, per chunk-pair and 512-row m-tile:
    S^T[n,m] = xnT_chunk.T @ xnT_band          (1 DoubleRow matmul per chunk)
    es = exp(S/256) -> fp8                     (ACT, [128,1024] instrs)
    Paug[m, 0:258] += es.T @ xa_pair           (DoubleRow; cols 256,257 = sumexp)
  finalize: out = 1.1*x_own - 0.1 * Pacc[:, perm]/Pacc[:, 256]
"""

import numpy as np

N, D, NCORES = 16384, 256, 8
M = N // NCORES          # 2048 rows per core
P = 128                  # partitions
SCALE = 0.1
LN16 = float(np.log(16.0))

_NC = None               # cached compiled Bass module


def build(n=N, m=M, compile=True):
    import concourse.bass as bass
    import concourse.tile as tile
    from concourse import bacc, mybir
    from contextlib import ExitStack

    F8 = mybir.dt.float8e4
    F16 = mybir.dt.float16
    F32 = mybir.dt.float32
    AF = mybir.ActivationFunctionType
    DR = mybir.MatmulPerfMode.DoubleRow

    nch = n // P             # n-chunks
    rch = m // P             # own row-chunks
    mt_w = min(512, m)       # m-tile width
    nmt = m // mt_w
    msub = mt_w // P
    TG = min(8, nch)         # chunks per setup group
    ngrp = nch // TG
    CP = min(64, nch)        # chunks per phase
    nph = nch // CP
    GPP = max(1, CP // TG)   # groups per phase
    LOOKAHEAD = 1

    # Keep Square/Ln/Exp on one ACT table set (avoids ~1.3us reloads).
    if not getattr(bacc, "_contranorm_act_patch", False):
        _orig_tables = bacc.get_activation_tables

        def _patched_tables(arch):
            keep = "natural_log_exp_and_others"
            return {k: (v if k == keep else set())
                    for k, v in _orig_tables(arch).items()}

        bacc.get_activation_tables = _patched_tables
        bacc._contranorm_act_patch = True

    nc = bacc.Bacc("TRN2", debug=False, num_devices=NCORES)
    x_d = nc.dram_tensor("x", (n, D), F32, kind="ExternalInput").ap()
    out_d = nc.dram_tensor("out", (m, D), F32, kind="ExternalOutput").ap()

    x_c = x_d.rearrange("(c p) d -> p c d", p=P)
    out_c = out_d.rearrange("(c p) d -> p c d", p=P)

    with tile.TileContext(nc) as tc, ExitStack() as ctx:
        big = ctx.enter_context(tc.tile_pool(name="big", bufs=1))
        ld = ctx.enter_context(tc.tile_pool(name="ld", bufs=2))
        ep = ctx.enter_context(tc.tile_pool(name="exp", bufs=4))
        fin = ctx.enter_context(tc.tile_pool(name="fin", bufs=2))
        sp = ctx.enter_context(tc.tile_pool(name="spsum", bufs=2, space="PSUM"))
        pp = ctx.enter_context(tc.tile_pool(name="ppsum", bufs=1, space="PSUM"))

        # persistent tiles
        xa = big.tile([P, nch, 257], F8)        # fp8 x, pair-interleaved + ones
        xnT = big.tile([P, 2, nch * P], F8)     # 16*x_norm^T, d-half planes
        xrs = big.tile([P, rch, D], F32)        # 1.1 * own rows (natural d order)
        pacc = big.tile([P, nmt * msub, 257], F32)
        ssq = big.tile([P, nch], F32)
        lnssq = big.tile([P, nch], F32)
        r32 = big.tile([P, nch], F32)           # 16/||x|| per row

        xts = {}
        xps = {}
        xtrs = {}

        def setup_piece(g, piece):
            c0 = g * TG
            if piece == 0:
                xt = xts[g] = ld.tile([P, TG, D], F32, name=f"xt{g}", tag="xt")
                nc.sync.dma_start(xt[:], x_c[:, c0:c0 + TG, :])
                if c0 < rch:  # own band: keep 1.1*x for finalize
                    nc.vector.tensor_scalar_mul(
                        xrs[:, c0:c0 + TG, :], xt[:], 1.1)
            elif piece == 1:
                xt = xts[g]
                for j in range(TG):
                    c = c0 + j
                    # pack fp8 pairs: col j2*2+b <- d = b*128+j2
                    nc.gpsimd.tensor_copy(
                        xa[:, c, 0:256].rearrange("p (j2 b) -> p b j2", b=2),
                        xt[:, j, :].rearrange("p (b j2) -> p b j2", b=2))
                nc.gpsimd.memset(xa[:, c0:c0 + TG, 256:257], 1.0)
            elif piece == 2:
                xt = xts[g]
                bn6 = ld.tile([P, TG, 6], F32, tag="bn6", name=f"bn6{g}")
                mv = ld.tile([P, TG, 2], F32, tag="mv", name=f"mv{g}")
                for j in range(TG):
                    nc.vector.bn_stats(bn6[:, j, :], xt[:, j, :])
                    nc.vector.bn_aggr(mv[:, j, :], bn6[:, j, :])
                # ssq/256 = mean^2 + var
                nc.vector.tensor_tensor(ssq[:, c0:c0 + TG], mv[:, :, 0],
                                        mv[:, :, 0], mybir.AluOpType.mult)
                nc.vector.tensor_tensor(ssq[:, c0:c0 + TG],
                                        ssq[:, c0:c0 + TG], mv[:, :, 1],
                                        mybir.AluOpType.add)
            elif piece == 3:
                # r = 16/sqrt(ssq) = exp(-0.5 * ln(ssq/256))
                nc.scalar.activation(lnssq[:, c0:c0 + TG],
                                     ssq[:, c0:c0 + TG], AF.Ln)
                nc.scalar.activation(r32[:, c0:c0 + TG],
                                     lnssq[:, c0:c0 + TG], AF.Exp,
                                     scale=-0.5)
            elif piece == 4:
                xt = xts.pop(g)
                xp = xps[g] = ld.tile([P, TG, 256], F8, name=f"xp{g}", tag="xp")
                for j in range(TG):
                    c = c0 + j
                    # 16*x_norm, fp8, pair-interleaved (per-partition scalar)
                    nc.vector.tensor_scalar_mul(
                        xp[:, j, :].rearrange("p (j2 b) -> p b j2", b=2),
                        xt[:, j, :].rearrange("p (b j2) -> p b j2", b=2),
                        r32[:, c:c + 1])
            elif piece == 5:
                xp = xps.pop(g)
                xtr = xtrs[g] = ld.tile([P, TG, 256], F8, name=f"xtr{g}",
                                        tag="xtr")
                nc.sync.dma_start_transpose(
                    xtr[:].bitcast(F16), xp[:].bitcast(F16))
            else:
                # de-interleave fp8 pairs into d-half planes of xnT
                xtr = xtrs.pop(g)
                src = xtr[:].rearrange("p c (j b) -> p b c j", b=2)
                band = slice(c0 * P, (c0 + TG) * P)
                nc.vector.tensor_copy(
                    xnT[:, 0, band].rearrange("p (c j) -> p c j", c=TG),
                    src[:, 0])
                nc.gpsimd.tensor_copy(
                    xnT[:, 1, band].rearrange("p (c j) -> p c j", c=TG),
                    src[:, 1])

        NPIECE = 7

        def setup_group(g):
            for piece in range(NPIECE):
                setup_piece(g, piece)

        def s_stationary(c):
            return xnT[:, :, c * P:(c + 1) * P]

        def s_moving(m0):
            return xnT[:, :, m0:m0 + mt_w]

        def phase(ph, setup_jobs=()):
            nonlocal pend
            jobs = list(setup_jobs)
            for mt in range(nmt):
                m0 = mt * mt_w
                paug = [pp.tile([P, 257], F32, tag=f"paug{ms}",
                                name=f"paug{ms}_{ph}_{mt}") for ms in range(msub)]
                for scn in range(CP // 2):
                    c0 = ph * CP + scn * 2
                    stp = sp.tile([P, 2, mt_w], F32, name=f"stp{ph}_{mt}_{scn}",
                                  tag="stp")
                    for j in range(2):
                        nc.tensor.matmul(stp[:, j, :], s_stationary(c0 + j),
                                         s_moving(m0), start=True, stop=True,
                                         perf_mode=DR)
                    es = ep.tile([P, 2, mt_w], F8, name=f"es{ph}_{mt}_{scn}",
                                 tag="es")
                    nc.scalar.activation(es[:], stp[:], AF.Exp, scale=1.0 / 256)
                    emit_pending()
                    pend = (es, c0, ph, paug, mt)
                nj = (len(jobs) + nmt - 1 - mt) // (nmt - mt) if jobs else 0
                for _ in range(nj):
                    g, piece = jobs.pop(0)
                    setup_piece(g, piece)

        def emit_pending():
            nonlocal pend
            if pend is None:
                return
            es, c0, ph, paug, mt = pend
            pend = None
            first = (c0 == ph * CP)
            last = (c0 == ph * CP + CP - 2)
            for ms in range(msub):
                nc.tensor.matmul(
                    paug[ms][:], es[:, :, ms * P:(ms + 1) * P],
                    xa[:, c0:c0 + 2, :], start=first, stop=last,
                    perf_mode=DR)
            if last:
                for ms in range(msub):
                    acc = pacc[:, mt * msub + ms, :]
                    if ph == 0:
                        nc.vector.tensor_copy(acc, paug[ms][:])
                    else:
                        nc.vector.tensor_add(acc, acc, paug[ms][:])

        def finalize():
            for rc in range(nmt * msub):
                r = fin.tile([P, 1], F32, tag="recip", name=f"r{rc}")
                nc.vector.reciprocal(r[:], pacc[:, rc, 256:257])
                rs = fin.tile([P, 1], F32, tag="rscaled", name=f"rs{rc}")
                nc.vector.tensor_scalar_mul(rs[:], r[:], -SCALE)
                t1 = fin.tile([P, D], F32, tag="scaledP", name=f"t1{rc}")
                # un-permute pair-interleaved cols: src (j*2+b) -> dst b*128+j
                nc.vector.tensor_scalar_mul(
                    t1[:].rearrange("p (b j) -> p b j", b=2),
                    pacc[:, rc, 0:256].rearrange("p (j b) -> p b j", b=2),
                    rs[:])
                ot = fin.tile([P, D], F32, tag="otile", name=f"ot{rc}")
                nc.vector.tensor_add(ot[:], xrs[:, rc, :], t1[:])
                nc.gpsimd.dma_start(out_c[:, rc, :], ot[:])

        pend = None
        prefill = min(ngrp, GPP * LOOKAHEAD)
        for g in range(prefill):
            setup_group(g)
        emitted = prefill
        for ph in range(nph):
            want = min(ngrp, GPP * (ph + 1 + LOOKAHEAD))
            jobs = [(g, piece) for g in range(emitted, want)
                    for piece in range(NPIECE)]
            emitted = want
            phase(ph, jobs)
        emit_pending()
        finalize()

    if compile:
        nc.compile()
    return nc


def _get_nc():
    global _NC
    if _NC is None:
        _NC = build()
    return _NC


def _run(x, trace=False):
    from concourse.bass_utils import run_bass_kernel_spmd

    x = np.ascontiguousarray(np.asarray(x, dtype=np.float32))
    assert x.shape == (N, D)
    in_maps = [{"x": np.ascontiguousarray(np.roll(x, -c * M, axis=0))}
               for c in range(NCORES)]
    res = run_bass_kernel_spmd(_get_nc(), in_maps, core_ids=list(range(NCORES)),
                               trace=trace)
    out = np.concatenate([res.results[c]["out"] for c in range(NCORES)], axis=0)
    return out, res


def kernel(x):
    out, _ = _run(x, trace=False)
    return out
